# revision 32
# baseline (speedup 1.0000x reference)
"""Trainium2 Bass kernel for one DPMM VB-EM iteration (M-step + E-step).

Strategy (data-parallel over rows, 8 cores):
  - Each core gets a 187500-row shard, zero-padded to 188928 = 128*1476 rows,
    laid out p-major: row n of the shard maps to (partition p, chunk i) with
    n = p*1476 + i. All DMAs are contiguous per partition.
  - Symmetric quadratic features per chunk: F = [x_d*x_e (d<=e, 10) | x (4)]
    = 14 per chunk; 9 chunks pack into a 128-col block (126 cols + shared
    ones col 126 + zero pad col 127). Off-diagonal quad coefficients in W
    are doubled to compensate.
  - NEFF A (stats): fp8 e4m3 inputs (quantization noise averages out over
    187k rows per cluster); DoubleRow fp8 matmuls contract 18 chunks per
    pass: stats += F''^T @ Phi'' with lhsT [128,2,128], rhs [128,2,144].
  - Host: sums the 8 partial stats, computes the variational M-step +
    E-step coefficient matrix W [128,144] in float64 (digamma, 4x4
    inverses, logdet), centers it per feature row (softmax invariance;
    keeps logits within +-0.1 so bf16 loses nothing), casts to bf16.
  - NEFF B (E-step): bf16; per 9-chunk group: PE-transpose F block
    [128,128], evac to SBUF (DVE), logits = FT^T @ W -> [128,144] f32
    PSUM, exp on ACT to bf16, DMA out bf16; row normalization on host.

Self-contained: hardcodes shapes for N=1500000, D=4, T=16, 8 cores.
"""
import os
import sys

os.environ.setdefault("CONCOURSE_KEEP_NRT", "1")
sys.path.insert(0, "/opt/trn_rl_repo")

from contextlib import ExitStack

import ml_dtypes
import numpy as np

import concourse.bass as bass
import concourse.tile as tile
from concourse import bacc
from concourse import mybir
from concourse.bass_utils import run_bass_kernel_spmd

F32 = mybir.dt.float32
BF16 = mybir.dt.bfloat16
FP8 = mybir.dt.float8e4
NP_BF16 = ml_dtypes.bfloat16
NP_FP8 = ml_dtypes.float8_e4m3

# ---------------- problem geometry ----------------
N_TOTAL = 1_500_000
D = 4
T = 16
NCORES = 8
RSH = N_TOTAL // NCORES          # rows per core (187500)
P = 128                          # partitions
M = 1476                         # chunks per core (p-major column count)
RPAD = P * M                     # padded rows per core (188928)
CF = 14                          # features per chunk (10 sym quad + 4 x)
GB = 9                           # chunks per B group / feature block
GA = 2 * GB                      # chunks per A group (DoubleRow pair)
NGB = M // GB                    # 164 B groups per core
NGA = M // GA                    # 82 A groups per core
NFEAT = 128                      # feature block: 9*14 + ones(126) + pad(127)
NW = GB * T                      # 144 matmul free size
TRI = (0, 1, 3, 6)               # col offset of quad pairs (d,e): TRI[e]+d

ALPHA_DP = 1e-3
LOG2 = float(np.log(2.0))

# Phi streaming tiles for NEFF A: chunks per DMA tile (multiples of GA)
PHI_TILES = [288, 288, 288, 288, 324]    # sums to 1476
# E-step supers: groups per PSUM slot (164 = 54*3 + 2); two 512-col (2KB
# bank) slots per lps tile so one exp instruction drains 2 supers; four
# 448-col (64B-aligned) e_t slots per out-DMA flush
SUPERS = [3] * 54 + [2]
LSLOT = 512                      # f32 cols per PSUM slot (one bank)
ESLOT = 448                      # bf16 cols per e_t slot (64B aligned)


def _build_feats(nc, fview, xview, nblk, eng_pair):
    """fview: [P, nblk, 9, >=14] feature block cols; xview: [P, nblk, 9, 4]
    matching chunk x values. Writes sym quad features (cols 0..9: pair
    (d,e) at TRI[e]+d) and x (cols 10..13), batched over all blocks.
    """
    e0, e1 = eng_pair
    e0.tensor_copy(fview[:, :, :, 10:14], xview)
    for e in range(D):
        src0 = xview[:, :, :, 0:e + 1]
        src1 = xview[:, :, :, e:e + 1].broadcast_to([P, nblk, GB, e + 1])
        eng = e0 if e % 2 == 0 else e1
        eng.tensor_mul(fview[:, :, :, TRI[e]:TRI[e] + e + 1], src0, src1)


def build_stats_nc(num_devices=NCORES, repeat=1):
    nc = bacc.Bacc("TRN2", target_bir_lowering=False, debug=False,
                   num_devices=num_devices)
    x = nc.dram_tensor("x", [RPAD, D], FP8, kind="ExternalInput")
    phi = nc.dram_tensor("phi", [RPAD, T], FP8, kind="ExternalInput")
    stats = nc.dram_tensor("stats", [NFEAT, NW], F32, kind="ExternalOutput")

    xr = x.ap().rearrange("(p i) d -> p i d", p=P)
    phir = phi.ap().rearrange("(p i) t -> p i t", p=P)

    with tile.TileContext(nc) as tc, ExitStack() as ctx:
        xpool = ctx.enter_context(tc.tile_pool(name="xp", bufs=1))
        fpool = ctx.enter_context(tc.tile_pool(name="fp", bufs=1))
        phipool = ctx.enter_context(tc.tile_pool(name="php", bufs=3))
        pspool = ctx.enter_context(
            tc.tile_pool(name="psp", bufs=1, space=bass.MemorySpace.PSUM))
        opool = ctx.enter_context(tc.tile_pool(name="op", bufs=1))

        x_sb = xpool.tile([P, M * D], FP8)
        xv = x_sb[:].rearrange("p (i d) -> p i d", d=D)
        nc.sync.dma_start(out=xv, in_=xr)

        # feature tile: 164 blocks (DoubleRow pair b = 2g+k) x 128 cols.
        # Host permutes chunks so device chunk j = 9b + c holds original
        # chunk 18g + 2c + k -- all device APs stay simple/regular.
        f6 = fpool.tile([P, 2 * NGA * NFEAT], FP8)
        fgk = f6[:].rearrange("p (b f) -> p b f", f=NFEAT)
        nc.vector.memset(fgk[:, :, 126:127], 1.0)
        nc.gpsimd.memset(fgk[:, :, 127:128], 0.0)
        fview = fgk[:, :, 0:GB * CF].rearrange("p b (c q) -> p b c q", q=CF)
        xg = xv.rearrange("p (b c) d -> p b c d", c=GB)
        _build_feats(nc, fview, xg, 2 * NGA, (nc.vector, nc.gpsimd))

        ps = pspool.tile([NFEAT, NW], F32)
        for _rep in range(repeat):
            gi = 0
            i0 = 0
            for si, cs in enumerate(PHI_TILES):
                pt = phipool.tile([P, cs * T], FP8, tag="pt")
                deng = nc.sync if si % 2 == 0 else nc.scalar
                deng.dma_start(
                    out=pt[:].rearrange("p (i t) -> p i t", t=T),
                    in_=phir[:, i0:i0 + cs, :])
                for gl in range(cs // GA):
                    lhsT = f6[:, (gi * 2) * NFEAT:(gi * 2 + 2) * NFEAT] \
                        .rearrange("p (k m) -> p k m", k=2)
                    rhs = pt[:, gl * GA * T:(gl + 1) * GA * T] \
                        .rearrange("p (k n) -> p k n", k=2)
                    nc.tensor.matmul(
                        ps[:], lhsT=lhsT, rhs=rhs,
                        perf_mode=mybir.MatmulPerfMode.DoubleRow,
                        start=(gi == 0), stop=(gi == NGA - 1))
                    gi += 1
                i0 += cs
            assert gi == NGA

        st_sb = opool.tile([NFEAT, NW], F32)
        nc.scalar.copy(st_sb[:], ps[:])
        nc.sync.dma_start(out=stats.ap(), in_=st_sb[:])
    nc.compile()
    return nc


ESTEP_STAGES = 4  # bench knob: 1=tr+evac, 2=+matmul, 3=+exp, 4=+dma (full)
FTPS_BUFS = 3
LPS_BUFS = 2
DVE_DRAIN_EVERY = 8  # if >0, drain d goes to DVE when d % DVE_DRAIN_EVERY
#                      == DVE_DRAIN_EVERY-1 (raw logits; host exps them)
SPLIT_DRAIN = False  # drain each super separately (contiguous reads)


def build_estep_nc(num_devices=NCORES, repeat=1):
    nc = bacc.Bacc("TRN2", target_bir_lowering=False, debug=False,
                   num_devices=num_devices)
    x = nc.dram_tensor("x", [RPAD, D], BF16, kind="ExternalInput")
    w = nc.dram_tensor("w", [NFEAT, NW], BF16, kind="ExternalInput")
    ident = nc.dram_tensor("ident", [P, P], BF16, kind="ExternalInput")
    phi_out = nc.dram_tensor("phi_out", [RPAD, T], BF16, kind="ExternalOutput")

    xr = x.ap().rearrange("(p i) d -> p i d", p=P)
    por = phi_out.ap().rearrange("(p i) t -> p i t", p=P)

    with tile.TileContext(nc) as tc, ExitStack() as ctx:
        xpool = ctx.enter_context(tc.tile_pool(name="xp", bufs=1))
        fpool = ctx.enter_context(tc.tile_pool(name="fp", bufs=1))
        cpool = ctx.enter_context(tc.tile_pool(name="cp", bufs=1))
        ftps_pool = ctx.enter_context(
            tc.tile_pool(name="ftps", bufs=FTPS_BUFS, space=bass.MemorySpace.PSUM))
        ftsb_pool = ctx.enter_context(tc.tile_pool(name="ftsb", bufs=3))
        lps_pool = ctx.enter_context(
            tc.tile_pool(name="lps", bufs=LPS_BUFS, space=bass.MemorySpace.PSUM))
        epool = ctx.enter_context(tc.tile_pool(name="ep", bufs=3))

        x_sb = xpool.tile([P, M * D], BF16)
        xv = x_sb[:].rearrange("p (i d) -> p i d", d=D)
        nc.sync.dma_start(out=xv, in_=xr)

        w_sb = cpool.tile([NFEAT, NW], BF16, tag="w")
        nc.sync.dma_start(out=w_sb[:], in_=w.ap())
        id_sb = cpool.tile([P, P], BF16, tag="id")
        nc.sync.dma_start(out=id_sb[:], in_=ident.ap())

        f9 = fpool.tile([P, NGB * NFEAT], BF16)
        fgv = f9[:].rearrange("p (g f) -> p g f", f=NFEAT)
        nc.vector.memset(fgv[:, :, 126:127], 1.0)
        nc.gpsimd.memset(fgv[:, :, 127:128], 0.0)
        fview = fgv[:, :, 0:GB * CF].rearrange("p g (c q) -> p g c q", q=CF)
        xg = xv.rearrange("p (g c) d -> p g c d", c=GB)
        _build_feats(nc, fview, xg, NGB, (nc.vector, nc.gpsimd))

        for _rep in range(repeat):
            g0 = 0
            c0 = 0
            nsup = len(SUPERS)
            for s, sg in enumerate(SUPERS):
                ft_ps = ftps_pool.tile([P, sg * P], BF16, tag="ftps")
                for k in range(sg):
                    nc.tensor.matmul(
                        ft_ps[:, k * P:(k + 1) * P],
                        lhsT=f9[:, (g0 + k) * NFEAT:(g0 + k + 1) * NFEAT],
                        rhs=id_sb[:], is_transpose=True, start=True, stop=True)
                ft_sb = ftsb_pool.tile([P, sg * P], BF16, tag="ftsb")
                nc.vector.tensor_copy(ft_sb[:], ft_ps[:])
                g0 += sg

                if ESTEP_STAGES < 2:
                    c0 += sg * GB
                    continue
                if s % 2 == 0:
                    l_ps = lps_pool.tile([P, 2 * LSLOT], F32, tag="lps")
                for k in range(sg):
                    nc.tensor.matmul(
                        l_ps[:, (s % 2) * LSLOT + k * NW:
                             (s % 2) * LSLOT + (k + 1) * NW],
                        lhsT=ft_sb[:, k * P:(k + 1) * P],
                        rhs=w_sb[:], start=True, stop=True)

                if ESTEP_STAGES < 3:
                    c0 += sg * GB
                    continue
                if s % 4 == 0:
                    e_t = epool.tile([P, 4 * ESLOT], BF16, tag="e")
                    dma_c0 = c0
                c0 += sg * GB
                if s % 2 == 1 or s == nsup - 1:
                    # drain the 1-2 supers in this lps tile in one op:
                    # exp on ACT, except every 3rd drain raw-copies the
                    # logits on GPSIMD (host applies exp there)
                    ns = 2 if s % 2 == 1 else 1
                    cw = sg * GB * T
                    sl0 = (s - ns + 1) % 4
                    if ns == 2:
                        src = l_ps[:].rearrange(
                            "p (m c) -> p m c", m=2)[:, :, 0:cw]
                        dst = e_t[:, sl0 * ESLOT:(sl0 + 2) * ESLOT].rearrange(
                            "p (m c) -> p m c", m=2)[:, :, 0:cw]
                    else:
                        src = l_ps[:, 0:cw]
                        dst = e_t[:, sl0 * ESLOT:sl0 * ESLOT + cw]
                    d = s // 2
                    on_dve = DVE_DRAIN_EVERY and d % DVE_DRAIN_EVERY == \
                        DVE_DRAIN_EVERY - 1
                    if SPLIT_DRAIN and ns == 2:
                        parts = [(l_ps[:, m * LSLOT:m * LSLOT + cw],
                                  e_t[:, (sl0 + m) * ESLOT:
                                      (sl0 + m) * ESLOT + cw])
                                 for m in range(2)]
                    else:
                        parts = [(src, dst)]
                    for psrc, pdst in parts:
                        if on_dve:
                            nc.vector.tensor_copy(pdst, psrc)
                        else:
                            nc.scalar.activation(
                                pdst, psrc, mybir.ActivationFunctionType.Exp)

                if ESTEP_STAGES < 4:
                    continue
                if s % 4 == 3 or s == nsup - 1:
                    filled = c0 - dma_c0
                    nfull = filled // (3 * GB)
                    rem = filled - nfull * 3 * GB
                    ev = e_t[:].rearrange("p (m c) -> p m c", m=4)
                    if nfull:
                        nc.sync.dma_start(
                            out=por[:, dma_c0:dma_c0 + nfull * 3 * GB, :],
                            in_=ev[:, 0:nfull, 0:3 * GB * T])
                    if rem:
                        nc.sync.dma_start(
                            out=por[:, dma_c0 + nfull * 3 * GB:c0, :],
                            in_=ev[:, nfull:nfull + 1, 0:rem * T])
            assert g0 == NGB
    nc.compile()
    return nc


# ---------------- host middle step ----------------

def _digamma(xx):
    xx = np.asarray(xx, dtype=np.float64)
    acc = np.zeros_like(xx)
    for k in range(8):
        acc += 1.0 / (xx + k)
    y = xx + 8.0
    y2 = 1.0 / (y * y)
    ser = np.log(y) - 0.5 / y - y2 * (1.0 / 12.0 - y2 * (1.0 / 120.0 - y2 / 252.0))
    return ser - acc


def _compute_W(stats_sum, priorMu, priorKappa, priorPsi, priorNu):
    """stats_sum [128,144] float64 -> centered W [128,144] float64."""
    Nk = np.zeros(T)
    Sx = np.zeros((D, T))
    Sxx = np.zeros((D, D, T))
    for c in range(GB):
        blk = stats_sum[CF * c:CF * c + CF, T * c:T * c + T]
        for e in range(D):
            for d in range(e + 1):
                Sxx[d, e] += blk[TRI[e] + d, :]
                if d != e:
                    Sxx[e, d] += blk[TRI[e] + d, :]
        Sx += blk[10:14, :]
        Nk += stats_sum[126, T * c:T * c + T]

    mu0 = np.asarray(priorMu, np.float64).reshape(D, 1)
    k0 = float(np.asarray(priorKappa).reshape(-1)[0])
    Psi0 = np.asarray(priorPsi, np.float64)
    nu0 = float(np.asarray(priorNu).reshape(-1)[0])

    g1 = 1.0 + Nk
    tail = np.cumsum(Nk[::-1])[::-1]
    g2 = ALPHA_DP + (tail - Nk)

    prior11 = Psi0 + k0 * (mu0 @ mu0.T)
    S = np.transpose(Sxx, (2, 0, 1))
    T12 = k0 * mu0 + Sx
    kappa = k0 + Nk
    mu = T12 / kappa[None, :]
    nu = Nk + nu0
    Psi = prior11[None] + S - kappa[:, None, None] * np.einsum('dt,et->tde', mu, mu)

    dg_sum = _digamma(g1 + g2)
    dg1 = _digamma(g1) - dg_sum
    dg2 = _digamma(g2) - dg_sum
    term2 = np.cumsum(dg2) - dg2

    Psi_inv = np.linalg.inv(Psi)
    sign, logdet = np.linalg.slogdet(Psi)
    Lam = nu[:, None, None] * Psi_inv
    eta2 = np.einsum('tde,et->td', Lam, mu)
    eta3 = -_digamma(0.5 * nu) - D * LOG2 + logdet
    quad = np.einsum('dt,tde,et->t', mu, Psi_inv, mu)
    eta4 = -0.5 * D / kappa - 0.5 * nu * quad

    const = dg1 + term2 - 0.5 * eta3 + eta4
    A = -0.5 * Lam                                   # [T, D, D] symmetric

    C = np.zeros((CF + 1, T), np.float64)
    for e in range(D):
        for d in range(e + 1):
            C[TRI[e] + d, :] = A[:, d, e] * (1.0 if d == e else 2.0)
    C[10:14, :] = eta2.T
    C[14, :] = const
    # center each coefficient row across clusters: shifts logits by a
    # per-sample constant -> softmax unchanged, logits become tiny
    C = C - C.mean(axis=1, keepdims=True)

    W = np.zeros((NFEAT, NW), np.float64)
    for c in range(GB):
        W[CF * c:CF * c + CF, T * c:T * c + T] = C[0:CF]
        W[126, T * c:T * c + T] = C[14]
    return W


# ---------------- top-level kernel ----------------

# A-side chunk permutation: device chunk j = 9*(2g+k) + c holds original
# chunk 18g + 2c + k (DoubleRow pairs adjacent original chunks).
_PERM_A = np.empty(M, np.int64)
for _b in range(2 * NGA):
    _g, _k = divmod(_b, 2)
    for _c in range(GB):
        _PERM_A[GB * _b + _c] = GA * _g + 2 * _c + _k

def _raw_chunk_mask():
    """Chunks whose E-step output is raw logits (DVE drains; host exps)."""
    m = np.zeros(M, bool)
    if DVE_DRAIN_EVERY:
        c = 0
        for s, sg in enumerate(SUPERS):
            if (s // 2) % DVE_DRAIN_EVERY == DVE_DRAIN_EVERY - 1:
                m[c:c + sg * GB] = True
            c += sg * GB
    return m

_CACHE = {}


def _get_ncs():
    if "stats" not in _CACHE:
        _CACHE["stats"] = build_stats_nc()
        _CACHE["estep"] = build_estep_nc()
    return _CACHE["stats"], _CACHE["estep"]


def kernel(data, Phi, priorMu, priorKappa, priorPsi, priorNu):
    data = np.asarray(data)
    Phi = np.asarray(Phi)
    nc_stats, nc_estep = _get_ncs()

    # shard + pad, p-major per core; bf16 for the E-step, fp8 (and
    # DoubleRow chunk-permuted) for stats
    xs, xs8, ps = [], [], []
    for c in range(NCORES):
        xc = np.zeros((RPAD, D), NP_BF16)
        pc = np.zeros((RPAD, T), NP_FP8)
        xc[:RSH] = data[c * RSH:(c + 1) * RSH].astype(NP_BF16)
        pc[:RSH] = Phi[c * RSH:(c + 1) * RSH].astype(NP_FP8)
        xs.append(xc)
        xs8.append(np.ascontiguousarray(
            xc.astype(NP_FP8).reshape(P, M, D)[:, _PERM_A].reshape(RPAD, D)))
        ps.append(np.ascontiguousarray(
            pc.reshape(P, M, T)[:, _PERM_A].reshape(RPAD, T)))

    in_maps = [{"x": xs8[c], "phi": ps[c]} for c in range(NCORES)]
    res_a = run_bass_kernel_spmd(nc_stats, in_maps, core_ids=list(range(NCORES)))
    stats_sum = np.zeros((NFEAT, NW), np.float64)
    for r in res_a.results:
        stats_sum += np.asarray(r["stats"], np.float64)

    W = _compute_W(stats_sum, priorMu, priorKappa, priorPsi, priorNu)
    Wb = np.ascontiguousarray(W.astype(NP_BF16))
    ident = np.ascontiguousarray(np.eye(P).astype(NP_BF16))

    in_maps_b = [{"x": xs[c], "w": Wb, "ident": ident} for c in range(NCORES)]
    res_b = run_bass_kernel_spmd(nc_estep, in_maps_b, core_ids=list(range(NCORES)))

    out = np.empty((N_TOTAL, T), np.float32)
    raw = _raw_chunk_mask()
    for c in range(NCORES):
        oc = res_b.results[c]["phi_out"].astype(np.float32).reshape(P, M, T)
        if raw.any():
            oc[:, raw] = np.exp(oc[:, raw])
        out[c * RSH:(c + 1) * RSH] = oc.reshape(RPAD, T)[:RSH]
    # normalize rows on host (exp of centered logits -> softmax)
    out /= out.sum(axis=1, keepdims=True)
    return out
